# revision 4
# baseline (speedup 1.0000x reference)
"""Trainium2 Bass kernel for a 2-layer GAT (nn_GAT_48524540510808).

Strategy (8 NeuronCores, SPMD):
- Nodes permuted by in-degree (desc) and dealt round-robin across cores:
  global rank k -> core k%8, local slot k//8; new node id = core*6250 + local.
- Per core, 49 blocks of 128 local nodes. Block b is processed in R_b rounds
  (R_b = degree at global rank 1024*b); round r slot p holds the r-th in-edge
  of local node 128*b+p (pad slots are masked with an additive -30000 on the
  attention logit). Segment-sum therefore becomes plain PSUM accumulation of
  per-round message tiles via an identity matmul (no scatter needed).
- Dense projections are data-parallel over nodes; per-layer tables
  (h | alpha_src) in bf16 are AllGathered so each core can gather rows of any
  src node. alpha_dst stays core-local in SBUF.
- Edge phase gathers 128 rows per round with one indirect DMA.
- Softmax over incoming edges is computed without segment-max
  (exp(leakyrelu(e)) = max(exp(e), exp(0.28 e)); logits are O(1) so direct
  exp is safe in fp32/bf16). Normalization happens after aggregation:
  out = (sum_e w_e h_src) / (sum_e w_e + eps).
- elu(z) = relu(z) + exp(min(z,0)) - 1; log_softmax via exp-with-accum + ln.
"""

import numpy as np
import ml_dtypes

import concourse.bass as bass
import concourse.mybir as mybir
import concourse.tile as tile
import concourse.bacc as bacc
import concourse.bass_utils as bass_utils

bf16 = ml_dtypes.bfloat16

N = 50000
E = 800000
IN_C = 512
MID = 8
HEADS = 8
OUT_C = 16
NEG_SLOPE = 0.28
EPS = 1e-16
NCORES = 8
PER = N // NCORES            # 6250
NBLK = (PER + 127) // 128    # 49
PER_PAD = NBLK * 128         # 6272
KIN = IN_C // 128            # 4

C1 = HEADS * MID             # 64  (layer-1 h channels)
T1C = C1 + HEADS             # 72  (tab1 row: h | alpha_src)
C2 = HEADS * OUT_C           # 128 (layer-2 h channels)
T2C = C2 + HEADS             # 136 (tab2 row: h2 | alpha_src2)

F32 = mybir.dt.float32
BF16 = mybir.dt.bfloat16
I32 = mybir.dt.int32
AF = mybir.ActivationFunctionType
ALU = mybir.AluOpType


# ---------------------------------------------------------------- host prep

def _host_prep(node_feature, adj_list, W1, att_src1, att_dst1, b1,
               W2, att_src2, att_dst2, b2):
    src = np.asarray(adj_list[0], np.int64)
    dst = np.asarray(adj_list[1], np.int64)

    deg = np.bincount(dst, minlength=N)
    order = np.argsort(-deg, kind="stable")          # rank -> node
    rank_of_node = np.empty(N, np.int64)
    rank_of_node[order] = np.arange(N)
    ranks = np.arange(N)
    new_of_rank = (ranks % NCORES) * PER + (ranks // NCORES)
    new_of_node = new_of_rank[rank_of_node]          # node -> new id
    node_of_new = np.empty(N, np.int64)
    node_of_new[new_of_node] = np.arange(N)

    deg_sorted = deg[order]
    R_blocks = [int(deg_sorted[1024 * b]) for b in range(NBLK)]
    R_blocks = [max(r, 1) for r in R_blocks]
    chunk0 = np.concatenate([[0], np.cumsum(R_blocks)]).astype(np.int64)
    TOT = int(chunk0[-1])

    ns, nd = new_of_node[src], new_of_node[dst]
    eorder = np.lexsort((ns, nd))
    nd_s, ns_s = nd[eorder], ns[eorder]
    grp_start = np.searchsorted(nd_s, np.arange(N), side="left")
    pos = np.arange(E) - grp_start[nd_s]
    core_e = nd_s // PER
    loc_e = nd_s % PER
    blk_e = loc_e // 128
    part_e = loc_e % 128
    chunk_e = chunk0[blk_e] + pos

    src_idx = np.zeros((NCORES, 128, TOT), np.int32)
    amask = np.full((NCORES, 128, TOT), -30000.0, np.float32)
    src_idx[core_e, part_e, chunk_e] = ns_s.astype(np.int32)
    amask[core_e, part_e, chunk_e] = 0.0

    # folded weights
    A1 = np.zeros((C1, 2 * HEADS), np.float32)
    for h in range(HEADS):
        A1[h * MID:(h + 1) * MID, h] = att_src1[h]
        A1[h * MID:(h + 1) * MID, HEADS + h] = att_dst1[h]
    W1e = np.concatenate([W1, W1 @ A1], axis=1).astype(bf16)     # [512, 80]
    A2 = np.zeros((C2, 2 * HEADS), np.float32)
    for h in range(HEADS):
        A2[h * OUT_C:(h + 1) * OUT_C, h] = att_src2[h]
        A2[h * OUT_C:(h + 1) * OUT_C, HEADS + h] = att_dst2[h]
    W2e = np.concatenate([W2, W2 @ A2], axis=1).astype(bf16)     # [64, 160]

    xp = np.asarray(node_feature)[node_of_new]                    # [N, 512]
    xT = np.ascontiguousarray(xp.T.astype(bf16))                  # [512, N]
    xT_slices = []
    for c in range(NCORES):
        s = np.zeros((IN_C, PER_PAD), bf16)
        s[:, :PER] = xT[:, c * PER:(c + 1) * PER]
        xT_slices.append(s)

    b1rep = np.tile(np.asarray(b1, np.float32)[None, :], (128, 1))
    b2rep = np.tile(np.asarray(b2, np.float32)[None, :], (128, 1))

    return dict(
        R_blocks=R_blocks, chunk0=chunk0, TOT=TOT,
        src_idx=src_idx, amask=amask,
        W1e=np.asarray(W1e), W2e=np.asarray(W2e),
        xT_slices=xT_slices, b1rep=b1rep, b2rep=b2rep,
        new_of_node=new_of_node,
    )


# ------------------------------------------------------------- bass program

def build_program(R_blocks, TOT):
    nc = bacc.Bacc("TRN2", num_devices=NCORES)

    t_xT = nc.dram_tensor("xT", [IN_C, PER_PAD], BF16, kind="ExternalInput")
    t_W1e = nc.dram_tensor("W1e", [IN_C, T1C + HEADS], BF16, kind="ExternalInput")
    t_W2e = nc.dram_tensor("W2e", [C1, T2C + HEADS], BF16, kind="ExternalInput")
    t_idx = nc.dram_tensor("sidx", [128, TOT], I32, kind="ExternalInput")
    t_amask = nc.dram_tensor("amask", [128, TOT], F32, kind="ExternalInput")
    t_b1 = nc.dram_tensor("b1rep", [128, C1], F32, kind="ExternalInput")
    t_b2 = nc.dram_tensor("b2rep", [128, OUT_C], F32, kind="ExternalInput")
    t_out = nc.dram_tensor("y", [PER, OUT_C], F32, kind="ExternalOutput")

    tab1_loc = nc.dram_tensor("tab1_loc", [PER, T1C], BF16)
    tab2_loc = nc.dram_tensor("tab2_loc", [PER, T2C], BF16)
    tab1 = nc.dram_tensor("tab1", [N, T1C], BF16, addr_space="Shared")
    tab2 = nc.dram_tensor("tab2", [N, T2C], BF16, addr_space="Shared")

    chunk0 = np.concatenate([[0], np.cumsum(R_blocks)]).astype(np.int64)

    with tile.TileContext(nc) as tc:
        _emit(tc, nc, R_blocks, chunk0, TOT,
              t_xT, t_W1e, t_W2e, t_idx, t_amask, t_b1, t_b2, t_out,
              tab1_loc, tab2_loc, tab1, tab2)
    nc.compile()
    return nc


def _emit(tc, nc, R_blocks, chunk0, TOT,
          t_xT, t_W1e, t_W2e, t_idx, t_amask, t_b1, t_b2, t_out,
          tab1_loc, tab2_loc, tab1, tab2):
    from concourse.masks import make_identity

    from contextlib import ExitStack
    ctx = ExitStack()
    st = ctx.enter_context(tc.tile_pool(name="static", bufs=1))
    xp_pool = ctx.enter_context(tc.tile_pool(name="xp", bufs=4))
    gp = ctx.enter_context(tc.tile_pool(name="gp", bufs=3))
    mp = ctx.enter_context(tc.tile_pool(name="mp", bufs=2))
    ep = ctx.enter_context(tc.tile_pool(name="ep", bufs=2))
    pp = ctx.enter_context(tc.tile_pool(name="pp", bufs=2, space="PSUM"))
    sp = ctx.enter_context(tc.tile_pool(name="sp", bufs=2))

    ident = st.tile([128, 128], BF16)
    make_identity(nc, ident[:])

    # static SBUF loads
    w1t = [st.tile([128, T1C + HEADS], BF16, name=f"w1_{k}", tag=f"w1_{k}") for k in range(KIN)]
    for k in range(KIN):
        nc.sync.dma_start(out=w1t[k][:], in_=t_W1e[k * 128:(k + 1) * 128, :])
    w2 = st.tile([C1, T2C + HEADS], BF16)
    nc.sync.dma_start(out=w2[:], in_=t_W2e[:, :])

    idx_sb = st.tile([128, TOT], I32)
    nc.sync.dma_start(out=idx_sb[:], in_=t_idx[:, :])
    am_sb = st.tile([128, TOT], F32)
    nc.sync.dma_start(out=am_sb[:], in_=t_amask[:, :])
    b1_sb = st.tile([128, C1], F32)
    nc.sync.dma_start(out=b1_sb[:], in_=t_b1[:, :])
    b2_sb = st.tile([128, OUT_C], F32)
    nc.sync.dma_start(out=b2_sb[:], in_=t_b2[:, :])

    ad1 = st.tile([128, NBLK * HEADS], BF16)
    ad2 = st.tile([128, NBLK * HEADS], BF16)

    # ---- phase 1: dense layer 1 (data-parallel over this core's nodes)
    for m in range(NBLK):
        nrows = min(128, PER - m * 128)
        pd = pp.tile([128, T1C + HEADS], F32, space="PSUM", tag="pd")
        for k in range(KIN):
            xt = xp_pool.tile([128, 128], BF16, tag="xt")
            nc.sync.dma_start(
                out=xt[:], in_=t_xT[k * 128:(k + 1) * 128, m * 128:m * 128 + 128])
            nc.tensor.matmul(out=pd[:], lhsT=xt[:], rhs=w1t[k][:],
                             start=(k == 0), stop=(k == KIN - 1))
        tb = sp.tile([128, T1C], BF16, tag="tb1")
        nc.vector.tensor_copy(out=tb[:], in_=pd[:, :T1C])
        nc.sync.dma_start(out=tab1_loc[m * 128:m * 128 + nrows, :], in_=tb[:nrows])
        nc.vector.tensor_copy(out=ad1[:, m * HEADS:(m + 1) * HEADS],
                              in_=pd[:, T1C:T1C + HEADS])

    # ---- phase 2: AllGather table 1
    nc.gpsimd.collective_compute(
        "AllGather", ALU.bypass,
        replica_groups=[list(range(NCORES))],
        ins=[tab1_loc.ap().opt()],
        outs=[tab1.ap().opt()],
    )

    # ---- phase 3: edge layer 1 (+fused dense layer 2 per block)
    for b in range(NBLK):
        R = R_blocks[b]
        c0 = int(chunk0[b])
        nrows = min(128, PER - b * 128)

        G = gp.tile([128, R * T1C], BF16, tag="G")
        for r in range(R):
            nc.gpsimd.indirect_dma_start(
                out=G[:, r * T1C:(r + 1) * T1C],
                out_offset=None,
                in_=tab1[:, :],
                in_offset=bass.IndirectOffsetOnAxis(
                    ap=idx_sb[:, c0 + r:c0 + r + 1], axis=0),
            )
        Gv = G[:].rearrange("p (r x) -> p r x", x=T1C)

        ea = ep.tile([128, R * HEADS], F32, tag="ea")
        nc.vector.tensor_tensor(
            out=ea[:].rearrange("p (r h) -> p r h", h=HEADS),
            in0=Gv[:, :, C1:T1C],
            in1=ad1[:, b * HEADS:(b + 1) * HEADS]
                .rearrange("p (o h) -> p o h", o=1).to_broadcast([128, R, HEADS]),
            op=ALU.add)
        eb = ep.tile([128, R * HEADS], F32, tag="eb")
        nc.vector.tensor_tensor(
            out=eb[:].rearrange("p (r h) -> p r h", h=HEADS),
            in0=ea[:].rearrange("p (r h) -> p r h", h=HEADS),
            in1=am_sb[:, c0:c0 + R]
                .rearrange("p (r o) -> p r o", o=1).to_broadcast([128, R, HEADS]),
            op=ALU.add)
        y1 = ep.tile([128, R * HEADS], BF16, tag="y1")
        nc.scalar.activation(y1[:], eb[:], AF.Exp)
        y2 = ep.tile([128, R * HEADS], BF16, tag="y2")
        nc.scalar.activation(y2[:], eb[:], AF.Exp, scale=NEG_SLOPE)

        msg = mp.tile([128, R * T1C], BF16, tag="msg")
        mv = msg[:].rearrange("p (r x) -> p r x", x=T1C)
        nc.vector.tensor_tensor(
            out=mv[:, :, C1:T1C],
            in0=y1[:].rearrange("p (r h) -> p r h", h=HEADS),
            in1=y2[:].rearrange("p (r h) -> p r h", h=HEADS),
            op=ALU.max)
        nc.vector.tensor_tensor(
            out=mv[:, :, 0:C1].rearrange("p r (h c) -> p r h c", c=MID),
            in0=Gv[:, :, 0:C1].rearrange("p r (h c) -> p r h c", c=MID),
            in1=mv[:, :, C1:T1C].rearrange("p r (h o) -> p r h o", o=1)
                .to_broadcast([128, R, HEADS, MID]),
            op=ALU.mult)

        pb = pp.tile([128, T1C], F32, space="PSUM", tag="pb")
        for r in range(R):
            nc.tensor.matmul(out=pb[:], lhsT=ident[:],
                             rhs=msg[:, r * T1C:(r + 1) * T1C],
                             start=(r == 0), stop=(r == R - 1))

        # post: normalize, bias, elu -> h1'' ; fused dense layer 2
        dn = sp.tile([128, HEADS], F32, tag="dn")
        nc.vector.tensor_scalar_add(out=dn[:], in0=pb[:, C1:T1C], scalar1=EPS)
        rc = sp.tile([128, HEADS], F32, tag="rc")
        nc.vector.reciprocal(out=rc[:], in_=dn[:])
        q = sp.tile([128, C1], F32, tag="q")
        nc.vector.tensor_tensor(
            out=q[:].rearrange("p (h c) -> p h c", c=MID),
            in0=pb[:, 0:C1].rearrange("p (h c) -> p h c", c=MID),
            in1=rc[:].rearrange("p (h o) -> p h o", o=1)
                .to_broadcast([128, HEADS, MID]),
            op=ALU.mult)
        z = sp.tile([128, C1], F32, tag="z")
        nc.vector.tensor_tensor(out=z[:], in0=q[:], in1=b1_sb[:], op=ALU.add)
        tmin = sp.tile([128, C1], F32, tag="tmin")
        nc.vector.tensor_scalar_min(out=tmin[:], in0=z[:], scalar1=0.0)
        u = sp.tile([128, C1], F32, tag="u")
        nc.scalar.activation(u[:], tmin[:], AF.Exp)
        d = sp.tile([128, C1], F32, tag="d")
        nc.vector.tensor_tensor(out=d[:], in0=z[:], in1=tmin[:], op=ALU.subtract)
        hp = sp.tile([128, C1], F32, tag="hp")
        nc.vector.tensor_tensor(out=hp[:], in0=d[:], in1=u[:], op=ALU.add)
        hb = sp.tile([128, C1], BF16, tag="hb")
        nc.vector.tensor_scalar_add(out=hb[:], in0=hp[:], scalar1=-1.0)

        pt = pp.tile([C1, 128], BF16, space="PSUM", tag="pt", bufs=1)
        nc.tensor.transpose(out=pt[:], in_=hb[:], identity=ident[:])
        hT = sp.tile([C1, 128], BF16, tag="hT")
        nc.vector.tensor_copy(out=hT[:], in_=pt[:])

        p2 = pp.tile([128, T2C + HEADS], F32, space="PSUM", tag="p2", bufs=1)
        nc.tensor.matmul(out=p2[:], lhsT=hT[:], rhs=w2[:], start=True, stop=True)
        t2 = sp.tile([128, T2C], BF16, tag="t2")
        nc.vector.tensor_copy(out=t2[:], in_=p2[:, :T2C])
        nc.sync.dma_start(out=tab2_loc[b * 128:b * 128 + nrows, :], in_=t2[:nrows])
        nc.vector.tensor_copy(out=ad2[:, b * HEADS:(b + 1) * HEADS],
                              in_=p2[:, T2C:T2C + HEADS])

    # ---- phase 4: AllGather table 2
    nc.gpsimd.collective_compute(
        "AllGather", ALU.bypass,
        replica_groups=[list(range(NCORES))],
        ins=[tab2_loc.ap().opt()],
        outs=[tab2.ap().opt()],
    )

    # ---- phase 5: edge layer 2 + head mean + log_softmax
    for b in range(NBLK):
        R = R_blocks[b]
        c0 = int(chunk0[b])
        nrows = min(128, PER - b * 128)

        G = gp.tile([128, R * T2C], BF16, tag="G2")
        for r in range(R):
            nc.gpsimd.indirect_dma_start(
                out=G[:, r * T2C:(r + 1) * T2C],
                out_offset=None,
                in_=tab2[:, :],
                in_offset=bass.IndirectOffsetOnAxis(
                    ap=idx_sb[:, c0 + r:c0 + r + 1], axis=0),
            )
        Gv = G[:].rearrange("p (r x) -> p r x", x=T2C)

        ea = ep.tile([128, R * HEADS], F32, tag="ea")
        nc.vector.tensor_tensor(
            out=ea[:].rearrange("p (r h) -> p r h", h=HEADS),
            in0=Gv[:, :, C2:T2C],
            in1=ad2[:, b * HEADS:(b + 1) * HEADS]
                .rearrange("p (o h) -> p o h", o=1).to_broadcast([128, R, HEADS]),
            op=ALU.add)
        eb = ep.tile([128, R * HEADS], F32, tag="eb")
        nc.vector.tensor_tensor(
            out=eb[:].rearrange("p (r h) -> p r h", h=HEADS),
            in0=ea[:].rearrange("p (r h) -> p r h", h=HEADS),
            in1=am_sb[:, c0:c0 + R]
                .rearrange("p (r o) -> p r o", o=1).to_broadcast([128, R, HEADS]),
            op=ALU.add)
        y1 = ep.tile([128, R * HEADS], BF16, tag="y1")
        nc.scalar.activation(y1[:], eb[:], AF.Exp)
        y2 = ep.tile([128, R * HEADS], BF16, tag="y2")
        nc.scalar.activation(y2[:], eb[:], AF.Exp, scale=NEG_SLOPE)

        msg = mp.tile([128, R * T2C], BF16, tag="msg2")
        mv = msg[:].rearrange("p (r x) -> p r x", x=T2C)
        nc.vector.tensor_tensor(
            out=mv[:, :, C2:T2C],
            in0=y1[:].rearrange("p (r h) -> p r h", h=HEADS),
            in1=y2[:].rearrange("p (r h) -> p r h", h=HEADS),
            op=ALU.max)
        nc.vector.tensor_tensor(
            out=mv[:, :, 0:C2].rearrange("p r (h c) -> p r h c", c=OUT_C),
            in0=Gv[:, :, 0:C2].rearrange("p r (h c) -> p r h c", c=OUT_C),
            in1=mv[:, :, C2:T2C].rearrange("p r (h o) -> p r h o", o=1)
                .to_broadcast([128, R, HEADS, OUT_C]),
            op=ALU.mult)

        pb = pp.tile([128, T2C], F32, space="PSUM", tag="pb")
        for r in range(R):
            nc.tensor.matmul(out=pb[:], lhsT=ident[:],
                             rhs=msg[:, r * T2C:(r + 1) * T2C],
                             start=(r == 0), stop=(r == R - 1))

        dn = sp.tile([128, HEADS], F32, tag="dn")
        nc.vector.tensor_scalar_add(out=dn[:], in0=pb[:, C2:T2C], scalar1=EPS)
        rc = sp.tile([128, HEADS], F32, tag="rc")
        nc.vector.reciprocal(out=rc[:], in_=dn[:])
        q2 = sp.tile([128, C2], F32, tag="q2")
        nc.vector.tensor_tensor(
            out=q2[:].rearrange("p (h c) -> p h c", c=OUT_C),
            in0=pb[:, 0:C2].rearrange("p (h c) -> p h c", c=OUT_C),
            in1=rc[:].rearrange("p (h o) -> p h o", o=1)
                .to_broadcast([128, HEADS, OUT_C]),
            op=ALU.mult)
        s1 = sp.tile([128, C2 // 2], F32, tag="s1")
        nc.vector.tensor_tensor(out=s1[:], in0=q2[:, :64], in1=q2[:, 64:], op=ALU.add)
        s2 = sp.tile([128, C2 // 4], F32, tag="s2")
        nc.vector.tensor_tensor(out=s2[:], in0=s1[:, :32], in1=s1[:, 32:], op=ALU.add)
        s3 = sp.tile([128, OUT_C], F32, tag="s3")
        nc.vector.tensor_tensor(out=s3[:], in0=s2[:, :16], in1=s2[:, 16:], op=ALU.add)
        o1 = sp.tile([128, OUT_C], F32, tag="o1")
        nc.vector.tensor_scalar_mul(out=o1[:], in0=s3[:], scalar1=1.0 / HEADS)
        o2 = sp.tile([128, OUT_C], F32, tag="o2")
        nc.vector.tensor_tensor(out=o2[:], in0=o1[:], in1=b2_sb[:], op=ALU.add)

        eo = sp.tile([128, OUT_C], F32, tag="eo")
        lsum = sp.tile([128, 1], F32, tag="lsum")
        nc.scalar.activation(eo[:], o2[:], AF.Exp, accum_out=lsum[:])
        lse = sp.tile([128, 1], F32, tag="lse")
        nc.scalar.activation(lse[:], lsum[:], AF.Ln)
        res = sp.tile([128, OUT_C], F32, tag="res")
        nc.vector.tensor_tensor(
            out=res[:], in0=o2[:],
            in1=lse[:].to_broadcast([128, OUT_C]), op=ALU.subtract)
        nc.sync.dma_start(out=t_out[b * 128:b * 128 + nrows, :], in_=res[:nrows])

    ctx.close()


# ------------------------------------------------------------------- driver

_CACHE = {}


def _get_program(key, R_blocks, TOT):
    if key not in _CACHE:
        _CACHE[key] = build_program(R_blocks, TOT)
    return _CACHE[key]


def kernel(node_feature, adj_list, W1, att_src1, att_dst1, b1,
           W2, att_src2, att_dst2, b2, _trace=False):
    prep = _host_prep(node_feature, adj_list, W1, att_src1, att_dst1, b1,
                      W2, att_src2, att_dst2, b2)
    R_blocks, TOT = prep["R_blocks"], prep["TOT"]
    nc = _get_program(tuple(R_blocks), R_blocks, TOT)

    in_maps = []
    for c in range(NCORES):
        in_maps.append({
            "xT": prep["xT_slices"][c],
            "W1e": prep["W1e"],
            "W2e": prep["W2e"],
            "sidx": prep["src_idx"][c],
            "amask": prep["amask"][c],
            "b1rep": prep["b1rep"],
            "b2rep": prep["b2rep"],
        })
    res = bass_utils.run_bass_kernel_spmd(
        nc, in_maps, list(range(NCORES)), trace=_trace)

    y_new = np.concatenate([res.results[c]["y"] for c in range(NCORES)], axis=0)
    out = y_new[prep["new_of_node"]]
    if _trace:
        kernel._last_result = res
    return np.ascontiguousarray(out.astype(np.float32))


# revision 8
# speedup vs baseline: 1.0374x; 1.0374x over previous
"""Trainium2 Bass kernel for a 2-layer GAT (nn_GAT_48524540510808).

Strategy (8 NeuronCores, SPMD):
- Nodes permuted by in-degree (desc) and dealt round-robin across cores:
  global rank k -> core k%8, local slot k//8; new node id = core*6250 + local.
- Per core, 49 blocks of 128 local nodes. Block b is processed in R_b rounds
  (R_b = degree at global rank 1024*b); round r slot p holds the r-th in-edge
  of local node 128*b+p (pad slots are masked with an additive -30000 on the
  attention logit). Segment-sum therefore becomes plain PSUM accumulation of
  per-round message tiles via an identity matmul (no scatter needed).
- Dense projections are data-parallel over nodes; per-layer tables
  (h | alpha_src) in bf16 are AllGathered so each core can gather rows of any
  src node. alpha_dst stays core-local in SBUF.
- Edge phase gathers 128 rows per round with one indirect DMA.
- Softmax over incoming edges is computed without segment-max
  (exp(leakyrelu(e)) = max(exp(e), exp(0.28 e)); logits are O(1) so direct
  exp is safe in fp32/bf16). Normalization happens after aggregation:
  out = (sum_e w_e h_src) / (sum_e w_e + eps).
- elu(z) = relu(z) + exp(min(z,0)) - 1; log_softmax via exp-with-accum + ln.
"""

import numpy as np
import ml_dtypes

import concourse.bass as bass
import concourse.mybir as mybir
import concourse.tile as tile
import concourse.bacc as bacc
import concourse.bass_utils as bass_utils

bf16 = ml_dtypes.bfloat16

N = 50000
E = 800000
IN_C = 512
MID = 8
HEADS = 8
OUT_C = 16
NEG_SLOPE = 0.28
EPS = 1e-16
NCORES = 8
PER = N // NCORES            # 6250
NBLK = (PER + 127) // 128    # 49
PER_PAD = NBLK * 128         # 6272
KIN = IN_C // 128            # 4

C1 = HEADS * MID             # 64  (layer-1 h channels)
T1C = C1 + HEADS             # 72  (tab1 row: h | alpha_src)
C2 = HEADS * OUT_C           # 128 (layer-2 h channels)
T2C = C2 + HEADS             # 136 (tab2 row: h2 | alpha_src2)

F32 = mybir.dt.float32
BF16 = mybir.dt.bfloat16
I32 = mybir.dt.int32
AF = mybir.ActivationFunctionType
ALU = mybir.AluOpType


# ---------------------------------------------------------------- host prep

def _host_prep(node_feature, adj_list, W1, att_src1, att_dst1, b1,
               W2, att_src2, att_dst2, b2):
    src = np.asarray(adj_list[0], np.int64)
    dst = np.asarray(adj_list[1], np.int64)

    deg = np.bincount(dst, minlength=N)
    order = np.argsort(-deg, kind="stable")          # rank -> node
    rank_of_node = np.empty(N, np.int64)
    rank_of_node[order] = np.arange(N)
    ranks = np.arange(N)
    new_of_rank = (ranks % NCORES) * PER_PAD + (ranks // NCORES)
    new_of_node = new_of_rank[rank_of_node]          # node -> new id

    deg_sorted = deg[order]
    R_blocks = [int(deg_sorted[1024 * b]) for b in range(NBLK)]
    R_blocks = [max(r, 1) for r in R_blocks]
    chunk0 = np.concatenate([[0], np.cumsum(R_blocks)]).astype(np.int64)
    TOT = int(chunk0[-1])

    ns, nd = new_of_node[src], new_of_node[dst]
    eorder = np.lexsort((ns, nd))
    nd_s, ns_s = nd[eorder], ns[eorder]
    grp_start = np.searchsorted(nd_s, np.arange(NCORES * PER_PAD), side="left")
    pos = np.arange(E) - grp_start[nd_s]
    core_e = nd_s // PER_PAD
    loc_e = nd_s % PER_PAD
    blk_e = loc_e // 128
    part_e = loc_e % 128
    chunk_e = chunk0[blk_e] + pos

    src_idx = np.zeros((NCORES, 128, TOT), np.int32)
    amask = np.full((NCORES, 128, TOT), -30000.0, np.float32)
    src_idx[core_e, part_e, chunk_e] = ns_s.astype(np.int32)
    amask[core_e, part_e, chunk_e] = 0.0

    # folded weights
    A1 = np.zeros((C1, 2 * HEADS), np.float32)
    for h in range(HEADS):
        A1[h * MID:(h + 1) * MID, h] = att_src1[h]
        A1[h * MID:(h + 1) * MID, HEADS + h] = att_dst1[h]
    W1e = np.concatenate([W1, W1 @ A1], axis=1).astype(bf16)     # [512, 80]
    A2 = np.zeros((C2, 2 * HEADS), np.float32)
    for h in range(HEADS):
        A2[h * OUT_C:(h + 1) * OUT_C, h] = att_src2[h]
        A2[h * OUT_C:(h + 1) * OUT_C, HEADS + h] = att_dst2[h]
    W2e = np.concatenate([W2, W2 @ A2], axis=1).astype(bf16)     # [64, 160]

    xf = np.asarray(node_feature)
    xT_slices = []
    for c in range(NCORES):
        nodes_c = order[c::NCORES]                   # local l -> node
        s = np.zeros((IN_C, PER_PAD), bf16)
        s[:, :PER] = xf[nodes_c].T.astype(bf16)
        xT_slices.append(s)

    b1rep = np.tile(np.asarray(b1, np.float32)[None, :], (128, 1))
    b2rep = np.tile(np.asarray(b2, np.float32)[None, :], (128, 1))

    return dict(
        R_blocks=R_blocks, chunk0=chunk0, TOT=TOT,
        src_idx=src_idx, amask=amask,
        W1e=np.asarray(W1e), W2e=np.asarray(W2e),
        xT_slices=xT_slices, b1rep=b1rep, b2rep=b2rep,
        new_of_node=new_of_node,
    )


# ------------------------------------------------------------- bass program

def build_program(R_blocks, TOT):
    nc = bacc.Bacc("TRN2", num_devices=NCORES)

    t_xT = nc.dram_tensor("xT", [IN_C, PER_PAD], BF16, kind="ExternalInput")
    t_W1e = nc.dram_tensor("W1e", [IN_C, T1C + HEADS], BF16, kind="ExternalInput")
    t_W2e = nc.dram_tensor("W2e", [C1, T2C + HEADS], BF16, kind="ExternalInput")
    t_idx = nc.dram_tensor("sidx", [128, TOT], I32, kind="ExternalInput")
    t_amask = nc.dram_tensor("amask", [128, TOT], F32, kind="ExternalInput")
    t_b1 = nc.dram_tensor("b1rep", [128, C1], F32, kind="ExternalInput")
    t_b2 = nc.dram_tensor("b2rep", [128, OUT_C], F32, kind="ExternalInput")
    t_out = nc.dram_tensor("y", [PER_PAD, OUT_C], F32, kind="ExternalOutput")

    tab1_loc = nc.dram_tensor("tab1_loc", [PER_PAD, T1C], BF16)
    tab2_loc = nc.dram_tensor("tab2_loc", [PER_PAD, T2C], BF16)
    tab1 = nc.dram_tensor("tab1", [NCORES * PER_PAD, T1C], BF16, addr_space="Shared")
    tab2 = nc.dram_tensor("tab2", [NCORES * PER_PAD, T2C], BF16, addr_space="Shared")

    chunk0 = np.concatenate([[0], np.cumsum(R_blocks)]).astype(np.int64)

    with tile.TileContext(nc) as tc:
        _emit(tc, nc, R_blocks, chunk0, TOT,
              t_xT, t_W1e, t_W2e, t_idx, t_amask, t_b1, t_b2, t_out,
              tab1_loc, tab2_loc, tab1, tab2)
    nc.compile()
    return nc


def _emit(tc, nc, R_blocks, chunk0, TOT,
          t_xT, t_W1e, t_W2e, t_idx, t_amask, t_b1, t_b2, t_out,
          tab1_loc, tab2_loc, tab1, tab2):
    from concourse.masks import make_identity

    from contextlib import ExitStack
    ctx = ExitStack()
    st = ctx.enter_context(tc.tile_pool(name="static", bufs=1))
    bigp = ctx.enter_context(tc.tile_pool(name="bigp", bufs=4))
    gp = ctx.enter_context(tc.tile_pool(name="gp", bufs=5))
    mp = ctx.enter_context(tc.tile_pool(name="mp", bufs=3))
    ep = ctx.enter_context(tc.tile_pool(name="ep", bufs=3))
    pp = ctx.enter_context(tc.tile_pool(name="pp", bufs=2, space="PSUM"))
    sp = ctx.enter_context(tc.tile_pool(name="sp", bufs=3))

    ident = st.tile([128, 128], BF16)
    make_identity(nc, ident[:])

    # static SBUF loads
    w1t = [st.tile([128, T1C + HEADS], BF16, name=f"w1_{k}", tag=f"w1_{k}") for k in range(KIN)]
    for k in range(KIN):
        nc.sync.dma_start(out=w1t[k][:], in_=t_W1e[k * 128:(k + 1) * 128, :])
    w2 = st.tile([C1, T2C + HEADS], BF16)
    nc.sync.dma_start(out=w2[:], in_=t_W2e[:, :])

    idx_sb = st.tile([128, TOT], I32)
    nc.sync.dma_start(out=idx_sb[:], in_=t_idx[:, :])
    am_sb = st.tile([128, TOT], F32)
    nc.sync.dma_start(out=am_sb[:], in_=t_amask[:, :])
    b1_sb = st.tile([128, C1], F32)
    nc.sync.dma_start(out=b1_sb[:], in_=t_b1[:, :])
    b2_sb = st.tile([128, OUT_C], F32)
    nc.sync.dma_start(out=b2_sb[:], in_=t_b2[:, :])

    ad1 = st.tile([128, NBLK * HEADS], BF16)
    ad2 = st.tile([128, NBLK * HEADS], BF16)
    tb_all = st.tile([128, NBLK * T1C], BF16)
    t2_all = st.tile([128, NBLK * T2C], BF16)
    ls_all = st.tile([128, NBLK], F32)
    o2_all = st.tile([128, NBLK * OUT_C], F32)

    # ---- phase 1: dense layer 1 (data-parallel over this core's nodes)
    xbig = []
    for k in range(KIN):
        xb = bigp.tile([128, PER_PAD], BF16, name=f"xb{k}", tag="bigshare")
        nc.sync.dma_start(out=xb[:], in_=t_xT[k * 128:(k + 1) * 128, :])
        xbig.append(xb)
    for m in range(NBLK):
        pd = pp.tile([128, T1C + HEADS], F32, space="PSUM", tag="pd")
        for k in range(KIN):
            nc.tensor.matmul(out=pd[:], lhsT=xbig[k][:, m * 128:m * 128 + 128],
                             rhs=w1t[k][:],
                             start=(k == 0), stop=(k == KIN - 1))
        nc.vector.tensor_copy(out=tb_all[:, m * T1C:(m + 1) * T1C], in_=pd[:, :T1C])
        nc.vector.tensor_copy(out=ad1[:, m * HEADS:(m + 1) * HEADS],
                              in_=pd[:, T1C:T1C + HEADS])
    nc.sync.dma_start(
        out=tab1_loc.ap().rearrange("(b p) c -> p b c", p=128),
        in_=tb_all[:].rearrange("p (b c) -> p b c", c=T1C))

    # ---- phase 2: AllGather table 1
    nc.gpsimd.collective_compute(
        "AllGather", ALU.bypass,
        replica_groups=[list(range(NCORES))],
        ins=[tab1_loc.ap().opt()],
        outs=[tab1.ap().opt()],
    )

    # ---- phase 3: edge layer 1 (+fused dense layer 2 per block)
    for b in range(NBLK):
        R = R_blocks[b]
        c0 = int(chunk0[b])
        nrows = min(128, PER - b * 128)

        G = gp.tile([128, R * T1C], BF16, tag="G")
        for r in range(R):
            nc.gpsimd.indirect_dma_start(
                out=G[:, r * T1C:(r + 1) * T1C],
                out_offset=None,
                in_=tab1[:, :],
                in_offset=bass.IndirectOffsetOnAxis(
                    ap=idx_sb[:, c0 + r:c0 + r + 1], axis=0),
            )
        Gv = G[:].rearrange("p (r x) -> p r x", x=T1C)

        ea = ep.tile([128, R * HEADS], F32, tag="ea")
        nc.vector.tensor_tensor(
            out=ea[:].rearrange("p (r h) -> p r h", h=HEADS),
            in0=Gv[:, :, C1:T1C],
            in1=ad1[:, b * HEADS:(b + 1) * HEADS]
                .rearrange("p (o h) -> p o h", o=1).to_broadcast([128, R, HEADS]),
            op=ALU.add)
        eb = ep.tile([128, R * HEADS], F32, tag="eb")
        nc.vector.tensor_tensor(
            out=eb[:].rearrange("p (r h) -> p r h", h=HEADS),
            in0=ea[:].rearrange("p (r h) -> p r h", h=HEADS),
            in1=am_sb[:, c0:c0 + R]
                .rearrange("p (r o) -> p r o", o=1).to_broadcast([128, R, HEADS]),
            op=ALU.add)
        y1 = ep.tile([128, R * HEADS], BF16, tag="y1")
        nc.scalar.activation(y1[:], eb[:], AF.Exp)
        y2 = ep.tile([128, R * HEADS], BF16, tag="y2")
        nc.scalar.activation(y2[:], eb[:], AF.Exp, scale=NEG_SLOPE)

        msg = mp.tile([128, R * T1C], BF16, tag="msg")
        mv = msg[:].rearrange("p (r x) -> p r x", x=T1C)
        nc.vector.tensor_tensor(
            out=mv[:, :, C1:T1C],
            in0=y1[:].rearrange("p (r h) -> p r h", h=HEADS),
            in1=y2[:].rearrange("p (r h) -> p r h", h=HEADS),
            op=ALU.max)
        nc.vector.tensor_tensor(
            out=mv[:, :, 0:C1].rearrange("p r (h c) -> p r h c", c=MID),
            in0=Gv[:, :, 0:C1].rearrange("p r (h c) -> p r h c", c=MID),
            in1=mv[:, :, C1:T1C].rearrange("p r (h o) -> p r h o", o=1)
                .to_broadcast([128, R, HEADS, MID]),
            op=ALU.mult)

        pb = pp.tile([128, T1C], F32, space="PSUM", tag="pb")
        for r in range(R):
            nc.tensor.matmul(out=pb[:], lhsT=ident[:],
                             rhs=msg[:, r * T1C:(r + 1) * T1C],
                             start=(r == 0), stop=(r == R - 1))

        # post: normalize, bias, elu -> h1'' ; fused dense layer 2
        dn = sp.tile([128, HEADS], F32, tag="dn")
        nc.vector.tensor_scalar_add(out=dn[:], in0=pb[:, C1:T1C], scalar1=EPS)
        rc = sp.tile([128, HEADS], F32, tag="rc")
        nc.vector.reciprocal(out=rc[:], in_=dn[:])
        q = sp.tile([128, C1], F32, tag="q")
        nc.vector.tensor_tensor(
            out=q[:].rearrange("p (h c) -> p h c", c=MID),
            in0=pb[:, 0:C1].rearrange("p (h c) -> p h c", c=MID),
            in1=rc[:].rearrange("p (h o) -> p h o", o=1)
                .to_broadcast([128, HEADS, MID]),
            op=ALU.mult)
        z = sp.tile([128, C1], F32, tag="z")
        nc.vector.tensor_tensor(out=z[:], in0=q[:], in1=b1_sb[:], op=ALU.add)
        tmin = sp.tile([128, C1], F32, tag="tmin")
        nc.vector.tensor_scalar_min(out=tmin[:], in0=z[:], scalar1=0.0)
        u = sp.tile([128, C1], F32, tag="u")
        nc.scalar.activation(u[:], tmin[:], AF.Exp)
        d = sp.tile([128, C1], F32, tag="d")
        nc.vector.tensor_tensor(out=d[:], in0=z[:], in1=tmin[:], op=ALU.subtract)
        hp = sp.tile([128, C1], F32, tag="hp")
        nc.vector.tensor_tensor(out=hp[:], in0=d[:], in1=u[:], op=ALU.add)
        hb = sp.tile([128, C1], BF16, tag="hb")
        nc.vector.tensor_scalar_add(out=hb[:], in0=hp[:], scalar1=-1.0)

        pt = pp.tile([C1, 128], BF16, space="PSUM", tag="pt", bufs=1)
        nc.tensor.transpose(out=pt[:], in_=hb[:], identity=ident[:])
        hT = sp.tile([C1, 128], BF16, tag="hT")
        nc.vector.tensor_copy(out=hT[:], in_=pt[:])

        p2 = pp.tile([128, T2C + HEADS], F32, space="PSUM", tag="p2", bufs=1)
        nc.tensor.matmul(out=p2[:], lhsT=hT[:], rhs=w2[:], start=True, stop=True)
        nc.vector.tensor_copy(out=t2_all[:, b * T2C:(b + 1) * T2C], in_=p2[:, :T2C])
        nc.vector.tensor_copy(out=ad2[:, b * HEADS:(b + 1) * HEADS],
                              in_=p2[:, T2C:T2C + HEADS])

    nc.sync.dma_start(
        out=tab2_loc.ap().rearrange("(b p) c -> p b c", p=128),
        in_=t2_all[:].rearrange("p (b c) -> p b c", c=T2C))

    # ---- phase 4: AllGather table 2
    nc.gpsimd.collective_compute(
        "AllGather", ALU.bypass,
        replica_groups=[list(range(NCORES))],
        ins=[tab2_loc.ap().opt()],
        outs=[tab2.ap().opt()],
    )

    # ---- phase 5: edge layer 2 + head mean + log_softmax
    for b in range(NBLK):
        R = R_blocks[b]
        c0 = int(chunk0[b])
        nrows = min(128, PER - b * 128)

        G = bigp.tile([128, R * T2C], BF16, tag="bigshare")
        for r in range(R):
            nc.gpsimd.indirect_dma_start(
                out=G[:, r * T2C:(r + 1) * T2C],
                out_offset=None,
                in_=tab2[:, :],
                in_offset=bass.IndirectOffsetOnAxis(
                    ap=idx_sb[:, c0 + r:c0 + r + 1], axis=0),
            )
        Gv = G[:].rearrange("p (r x) -> p r x", x=T2C)

        ea = ep.tile([128, R * HEADS], F32, tag="ea")
        nc.vector.tensor_tensor(
            out=ea[:].rearrange("p (r h) -> p r h", h=HEADS),
            in0=Gv[:, :, C2:T2C],
            in1=ad2[:, b * HEADS:(b + 1) * HEADS]
                .rearrange("p (o h) -> p o h", o=1).to_broadcast([128, R, HEADS]),
            op=ALU.add)
        eb = ep.tile([128, R * HEADS], F32, tag="eb")
        nc.vector.tensor_tensor(
            out=eb[:].rearrange("p (r h) -> p r h", h=HEADS),
            in0=ea[:].rearrange("p (r h) -> p r h", h=HEADS),
            in1=am_sb[:, c0:c0 + R]
                .rearrange("p (r o) -> p r o", o=1).to_broadcast([128, R, HEADS]),
            op=ALU.add)
        y1 = ep.tile([128, R * HEADS], BF16, tag="y1")
        nc.scalar.activation(y1[:], eb[:], AF.Exp)
        y2 = ep.tile([128, R * HEADS], BF16, tag="y2")
        nc.scalar.activation(y2[:], eb[:], AF.Exp, scale=NEG_SLOPE)

        msg = mp.tile([128, R * T2C], BF16, tag="msg2")
        mv = msg[:].rearrange("p (r x) -> p r x", x=T2C)
        nc.vector.tensor_tensor(
            out=mv[:, :, C2:T2C],
            in0=y1[:].rearrange("p (r h) -> p r h", h=HEADS),
            in1=y2[:].rearrange("p (r h) -> p r h", h=HEADS),
            op=ALU.max)
        nc.vector.tensor_tensor(
            out=mv[:, :, 0:C2].rearrange("p r (h c) -> p r h c", c=OUT_C),
            in0=Gv[:, :, 0:C2].rearrange("p r (h c) -> p r h c", c=OUT_C),
            in1=mv[:, :, C2:T2C].rearrange("p r (h o) -> p r h o", o=1)
                .to_broadcast([128, R, HEADS, OUT_C]),
            op=ALU.mult)

        pb = pp.tile([128, T2C], F32, space="PSUM", tag="pb")
        for r in range(R):
            nc.tensor.matmul(out=pb[:], lhsT=ident[:],
                             rhs=msg[:, r * T2C:(r + 1) * T2C],
                             start=(r == 0), stop=(r == R - 1))

        dn = sp.tile([128, HEADS], F32, tag="dn")
        nc.vector.tensor_scalar_add(out=dn[:], in0=pb[:, C2:T2C], scalar1=EPS)
        rc = sp.tile([128, HEADS], F32, tag="rc")
        nc.vector.reciprocal(out=rc[:], in_=dn[:])
        q2 = sp.tile([128, C2], F32, tag="q2")
        nc.vector.tensor_tensor(
            out=q2[:].rearrange("p (h c) -> p h c", c=OUT_C),
            in0=pb[:, 0:C2].rearrange("p (h c) -> p h c", c=OUT_C),
            in1=rc[:].rearrange("p (h o) -> p h o", o=1)
                .to_broadcast([128, HEADS, OUT_C]),
            op=ALU.mult)
        s1 = sp.tile([128, C2 // 2], F32, tag="s1")
        nc.vector.tensor_tensor(out=s1[:], in0=q2[:, :64], in1=q2[:, 64:], op=ALU.add)
        s2 = sp.tile([128, C2 // 4], F32, tag="s2")
        nc.vector.tensor_tensor(out=s2[:], in0=s1[:, :32], in1=s1[:, 32:], op=ALU.add)
        s3 = sp.tile([128, OUT_C], F32, tag="s3")
        nc.vector.tensor_tensor(out=s3[:], in0=s2[:, :16], in1=s2[:, 16:], op=ALU.add)
        o1 = sp.tile([128, OUT_C], F32, tag="o1")
        nc.vector.tensor_scalar_mul(out=o1[:], in0=s3[:], scalar1=1.0 / HEADS)
        o2 = sp.tile([128, OUT_C], F32, tag="o2")
        nc.vector.tensor_tensor(out=o2[:], in0=o1[:], in1=b2_sb[:], op=ALU.add)

        eo = sp.tile([128, OUT_C], F32, tag="eo")
        nc.scalar.activation(eo[:], o2[:], AF.Exp,
                             accum_out=ls_all[:, b:b + 1])
        nc.vector.tensor_copy(out=o2_all[:, b * OUT_C:(b + 1) * OUT_C], in_=o2[:])

    ln_all = st.tile([128, NBLK], F32)
    nc.scalar.activation(ln_all[:], ls_all[:], AF.Ln)
    res_all = st.tile([128, NBLK * OUT_C], F32)
    nc.vector.tensor_tensor(
        out=res_all[:].rearrange("p (b c) -> p b c", c=OUT_C),
        in0=o2_all[:].rearrange("p (b c) -> p b c", c=OUT_C),
        in1=ln_all[:].rearrange("p (b o) -> p b o", o=1)
            .to_broadcast([128, NBLK, OUT_C]),
        op=ALU.subtract)
    nc.sync.dma_start(
        out=t_out.ap().rearrange("(b p) c -> p b c", p=128),
        in_=res_all[:].rearrange("p (b c) -> p b c", c=OUT_C))

    ctx.close()


# ------------------------------------------------------------------- driver

_CACHE = {}


def _get_program(key, R_blocks, TOT):
    if key not in _CACHE:
        _CACHE[key] = build_program(R_blocks, TOT)
    return _CACHE[key]


def kernel(node_feature, adj_list, W1, att_src1, att_dst1, b1,
           W2, att_src2, att_dst2, b2, _trace=False):
    prep = _host_prep(node_feature, adj_list, W1, att_src1, att_dst1, b1,
                      W2, att_src2, att_dst2, b2)
    R_blocks, TOT = prep["R_blocks"], prep["TOT"]
    nc = _get_program(tuple(R_blocks), R_blocks, TOT)

    in_maps = []
    for c in range(NCORES):
        in_maps.append({
            "xT": prep["xT_slices"][c],
            "W1e": prep["W1e"],
            "W2e": prep["W2e"],
            "sidx": prep["src_idx"][c],
            "amask": prep["amask"][c],
            "b1rep": prep["b1rep"],
            "b2rep": prep["b2rep"],
        })
    res = bass_utils.run_bass_kernel_spmd(
        nc, in_maps, list(range(NCORES)), trace=_trace)

    y_new = np.concatenate([res.results[c]["y"] for c in range(NCORES)], axis=0)
    out = y_new[prep["new_of_node"]]
    if _trace:
        kernel._last_result = res
    return np.ascontiguousarray(out.astype(np.float32))


# revision 9
# speedup vs baseline: 1.0430x; 1.0054x over previous
"""Trainium2 Bass kernel for a 2-layer GAT (nn_GAT_48524540510808).

Strategy (8 NeuronCores, SPMD):
- Nodes permuted by in-degree (desc) and dealt round-robin across cores:
  global rank k -> core k%8, local slot k//8; new node id = core*6250 + local.
- Per core, 49 blocks of 128 local nodes. Block b is processed in R_b rounds
  (R_b = degree at global rank 1024*b); round r slot p holds the r-th in-edge
  of local node 128*b+p (pad slots are masked with an additive -30000 on the
  attention logit). Segment-sum therefore becomes plain PSUM accumulation of
  per-round message tiles via an identity matmul (no scatter needed).
- Dense projections are data-parallel over nodes; per-layer tables
  (h | alpha_src) in bf16 are AllGathered so each core can gather rows of any
  src node. alpha_dst stays core-local in SBUF.
- Edge phase gathers 128 rows per round with one indirect DMA.
- Softmax over incoming edges is computed without segment-max
  (exp(leakyrelu(e)) = max(exp(e), exp(0.28 e)); logits are O(1) so direct
  exp is safe in fp32/bf16). Normalization happens after aggregation:
  out = (sum_e w_e h_src) / (sum_e w_e + eps).
- elu(z) = relu(z) + exp(min(z,0)) - 1; log_softmax via exp-with-accum + ln.
"""

import numpy as np
import ml_dtypes

import concourse.bass as bass
import concourse.mybir as mybir
import concourse.tile as tile
import concourse.bacc as bacc
import concourse.bass_utils as bass_utils

bf16 = ml_dtypes.bfloat16

N = 50000
E = 800000
IN_C = 512
MID = 8
HEADS = 8
OUT_C = 16
NEG_SLOPE = 0.28
EPS = 1e-16
NCORES = 8
PER = N // NCORES            # 6250
NBLK = (PER + 127) // 128    # 49
PER_PAD = NBLK * 128         # 6272
KIN = IN_C // 128            # 4

C1 = HEADS * MID             # 64  (layer-1 h channels)
T1C = C1 + HEADS             # 72  (tab1 row: h | alpha_src)
C2 = HEADS * OUT_C           # 128 (layer-2 h channels)
T2C = C2 + HEADS             # 136 (tab2 row: h2 | alpha_src2)

F32 = mybir.dt.float32
BF16 = mybir.dt.bfloat16
I32 = mybir.dt.int32
AF = mybir.ActivationFunctionType
ALU = mybir.AluOpType


# ---------------------------------------------------------------- host prep

def _host_prep(node_feature, adj_list, W1, att_src1, att_dst1, b1,
               W2, att_src2, att_dst2, b2):
    src = np.asarray(adj_list[0], np.int64)
    dst = np.asarray(adj_list[1], np.int64)

    deg = np.bincount(dst, minlength=N)
    order = np.argsort(-deg, kind="stable")          # rank -> node
    rank_of_node = np.empty(N, np.int64)
    rank_of_node[order] = np.arange(N)
    ranks = np.arange(N)
    new_of_rank = (ranks % NCORES) * PER_PAD + (ranks // NCORES)
    new_of_node = new_of_rank[rank_of_node]          # node -> new id

    deg_sorted = deg[order]
    R_blocks = [int(deg_sorted[1024 * b]) for b in range(NBLK)]
    R_blocks = [max(r, 1) for r in R_blocks]
    chunk0 = np.concatenate([[0], np.cumsum(R_blocks)]).astype(np.int64)
    TOT = int(chunk0[-1])

    ns, nd = new_of_node[src], new_of_node[dst]
    eorder = np.lexsort((ns, nd))
    nd_s, ns_s = nd[eorder], ns[eorder]
    grp_start = np.searchsorted(nd_s, np.arange(NCORES * PER_PAD), side="left")
    pos = np.arange(E) - grp_start[nd_s]
    core_e = nd_s // PER_PAD
    loc_e = nd_s % PER_PAD
    blk_e = loc_e // 128
    part_e = loc_e % 128
    chunk_e = chunk0[blk_e] + pos

    src_idx = np.zeros((NCORES, 128, TOT), np.int32)
    amask = np.full((NCORES, 128, TOT), -30000.0, np.float32)
    src_idx[core_e, part_e, chunk_e] = ns_s.astype(np.int32)
    amask[core_e, part_e, chunk_e] = 0.0

    # folded weights
    A1 = np.zeros((C1, 2 * HEADS), np.float32)
    for h in range(HEADS):
        A1[h * MID:(h + 1) * MID, h] = att_src1[h]
        A1[h * MID:(h + 1) * MID, HEADS + h] = att_dst1[h]
    W1e = np.concatenate([W1, W1 @ A1], axis=1).astype(bf16)     # [512, 80]
    A2 = np.zeros((C2, 2 * HEADS), np.float32)
    for h in range(HEADS):
        A2[h * OUT_C:(h + 1) * OUT_C, h] = att_src2[h]
        A2[h * OUT_C:(h + 1) * OUT_C, HEADS + h] = att_dst2[h]
    W2e = np.concatenate([W2, W2 @ A2], axis=1).astype(bf16)     # [64, 160]

    xf = np.asarray(node_feature)
    xT_slices = []
    for c in range(NCORES):
        nodes_c = order[c::NCORES]                   # local l -> node
        s = np.zeros((IN_C, PER_PAD), bf16)
        s[:, :PER] = xf[nodes_c].T.astype(bf16)
        xT_slices.append(s)

    b1rep = np.tile(np.asarray(b1, np.float32)[None, :], (128, 1))
    b2rep = np.tile(np.asarray(b2, np.float32)[None, :], (128, 1))

    return dict(
        R_blocks=R_blocks, chunk0=chunk0, TOT=TOT,
        src_idx=src_idx, amask=amask,
        W1e=np.asarray(W1e), W2e=np.asarray(W2e),
        xT_slices=xT_slices, b1rep=b1rep, b2rep=b2rep,
        new_of_node=new_of_node,
    )


# ------------------------------------------------------------- bass program

def build_program(R_blocks, TOT):
    nc = bacc.Bacc("TRN2", num_devices=NCORES)

    t_xT = nc.dram_tensor("xT", [IN_C, PER_PAD], BF16, kind="ExternalInput")
    t_W1e = nc.dram_tensor("W1e", [IN_C, T1C + HEADS], BF16, kind="ExternalInput")
    t_W2e = nc.dram_tensor("W2e", [C1, T2C + HEADS], BF16, kind="ExternalInput")
    t_idx = nc.dram_tensor("sidx", [128, TOT], I32, kind="ExternalInput")
    t_amask = nc.dram_tensor("amask", [128, TOT], F32, kind="ExternalInput")
    t_b1 = nc.dram_tensor("b1rep", [128, C1], F32, kind="ExternalInput")
    t_b2 = nc.dram_tensor("b2rep", [128, OUT_C], F32, kind="ExternalInput")
    t_out = nc.dram_tensor("y", [PER_PAD, OUT_C], F32, kind="ExternalOutput")

    tab1_loc = nc.dram_tensor("tab1_loc", [PER_PAD, T1C], BF16)
    tab2_loc = nc.dram_tensor("tab2_loc", [PER_PAD, T2C], BF16)
    tab1 = nc.dram_tensor("tab1", [NCORES * PER_PAD, T1C], BF16, addr_space="Shared")
    tab2 = nc.dram_tensor("tab2", [NCORES * PER_PAD, T2C], BF16, addr_space="Shared")

    chunk0 = np.concatenate([[0], np.cumsum(R_blocks)]).astype(np.int64)

    with tile.TileContext(nc) as tc:
        _emit(tc, nc, R_blocks, chunk0, TOT,
              t_xT, t_W1e, t_W2e, t_idx, t_amask, t_b1, t_b2, t_out,
              tab1_loc, tab2_loc, tab1, tab2)
    nc.compile()
    return nc


def _emit(tc, nc, R_blocks, chunk0, TOT,
          t_xT, t_W1e, t_W2e, t_idx, t_amask, t_b1, t_b2, t_out,
          tab1_loc, tab2_loc, tab1, tab2):
    from concourse.masks import make_identity

    from contextlib import ExitStack
    ctx = ExitStack()
    st = ctx.enter_context(tc.tile_pool(name="static", bufs=1))
    bigp = ctx.enter_context(tc.tile_pool(name="bigp", bufs=5))
    gp = ctx.enter_context(tc.tile_pool(name="gp", bufs=7))
    mp = ctx.enter_context(tc.tile_pool(name="mp", bufs=3))
    ep = ctx.enter_context(tc.tile_pool(name="ep", bufs=3))
    pp = ctx.enter_context(tc.tile_pool(name="pp", bufs=2, space="PSUM"))
    sp = ctx.enter_context(tc.tile_pool(name="sp", bufs=3))

    ident = st.tile([128, 128], BF16)
    make_identity(nc, ident[:])

    # static SBUF loads
    w1t = [st.tile([128, T1C + HEADS], BF16, name=f"w1_{k}", tag=f"w1_{k}") for k in range(KIN)]
    for k in range(KIN):
        nc.sync.dma_start(out=w1t[k][:], in_=t_W1e[k * 128:(k + 1) * 128, :])
    w2 = st.tile([C1, T2C + HEADS], BF16)
    nc.sync.dma_start(out=w2[:], in_=t_W2e[:, :])

    idx_sb = st.tile([128, TOT], I32)
    nc.sync.dma_start(out=idx_sb[:], in_=t_idx[:, :])
    am_sb = st.tile([128, TOT], F32)
    nc.sync.dma_start(out=am_sb[:], in_=t_amask[:, :])
    b1_sb = st.tile([128, C1], F32)
    nc.sync.dma_start(out=b1_sb[:], in_=t_b1[:, :])
    b2_sb = st.tile([128, OUT_C], F32)
    nc.sync.dma_start(out=b2_sb[:], in_=t_b2[:, :])

    ad1 = st.tile([128, NBLK * HEADS], BF16)
    ad2 = st.tile([128, NBLK * HEADS], BF16)
    tb_all = st.tile([128, NBLK * T1C], BF16)
    ls_all = st.tile([128, NBLK], F32)
    o2_all = st.tile([128, NBLK * OUT_C], F32)

    # ---- phase 1: dense layer 1 (data-parallel over this core's nodes)
    xbig = []
    for k in range(KIN):
        xb = bigp.tile([128, PER_PAD], BF16, name=f"xb{k}", tag="bigshare")
        nc.sync.dma_start(out=xb[:], in_=t_xT[k * 128:(k + 1) * 128, :])
        xbig.append(xb)
    for m in range(NBLK):
        pd = pp.tile([128, T1C + HEADS], F32, space="PSUM", tag="pd")
        for k in range(KIN):
            nc.tensor.matmul(out=pd[:], lhsT=xbig[k][:, m * 128:m * 128 + 128],
                             rhs=w1t[k][:],
                             start=(k == 0), stop=(k == KIN - 1))
        nc.vector.tensor_copy(out=tb_all[:, m * T1C:(m + 1) * T1C], in_=pd[:, :T1C])
        nc.vector.tensor_copy(out=ad1[:, m * HEADS:(m + 1) * HEADS],
                              in_=pd[:, T1C:T1C + HEADS])
    nc.sync.dma_start(
        out=tab1_loc.ap().rearrange("(b p) c -> p b c", p=128),
        in_=tb_all[:].rearrange("p (b c) -> p b c", c=T1C))

    # ---- phase 2: AllGather table 1
    nc.gpsimd.collective_compute(
        "AllGather", ALU.bypass,
        replica_groups=[list(range(NCORES))],
        ins=[tab1_loc.ap().opt()],
        outs=[tab1.ap().opt()],
    )

    # ---- phase 3: edge layer 1 (+fused dense layer 2 per block)
    for b in range(NBLK):
        R = R_blocks[b]
        c0 = int(chunk0[b])
        nrows = min(128, PER - b * 128)

        G = gp.tile([128, R * T1C], BF16, tag="G")
        for r in range(R):
            nc.gpsimd.indirect_dma_start(
                out=G[:, r * T1C:(r + 1) * T1C],
                out_offset=None,
                in_=tab1[:, :],
                in_offset=bass.IndirectOffsetOnAxis(
                    ap=idx_sb[:, c0 + r:c0 + r + 1], axis=0),
            )
        Gv = G[:].rearrange("p (r x) -> p r x", x=T1C)

        ea = ep.tile([128, R * HEADS], F32, tag="ea")
        nc.vector.tensor_tensor(
            out=ea[:].rearrange("p (r h) -> p r h", h=HEADS),
            in0=Gv[:, :, C1:T1C],
            in1=ad1[:, b * HEADS:(b + 1) * HEADS]
                .rearrange("p (o h) -> p o h", o=1).to_broadcast([128, R, HEADS]),
            op=ALU.add)
        eb = ep.tile([128, R * HEADS], F32, tag="eb")
        nc.vector.tensor_tensor(
            out=eb[:].rearrange("p (r h) -> p r h", h=HEADS),
            in0=ea[:].rearrange("p (r h) -> p r h", h=HEADS),
            in1=am_sb[:, c0:c0 + R]
                .rearrange("p (r o) -> p r o", o=1).to_broadcast([128, R, HEADS]),
            op=ALU.add)
        y1 = ep.tile([128, R * HEADS], BF16, tag="y1")
        nc.scalar.activation(y1[:], eb[:], AF.Exp)
        y2 = ep.tile([128, R * HEADS], BF16, tag="y2")
        nc.scalar.activation(y2[:], eb[:], AF.Exp, scale=NEG_SLOPE)

        msg = mp.tile([128, R * T1C], BF16, tag="msg")
        mv = msg[:].rearrange("p (r x) -> p r x", x=T1C)
        nc.vector.tensor_tensor(
            out=mv[:, :, C1:T1C],
            in0=y1[:].rearrange("p (r h) -> p r h", h=HEADS),
            in1=y2[:].rearrange("p (r h) -> p r h", h=HEADS),
            op=ALU.max)
        nc.vector.tensor_tensor(
            out=mv[:, :, 0:C1].rearrange("p r (h c) -> p r h c", c=MID),
            in0=Gv[:, :, 0:C1].rearrange("p r (h c) -> p r h c", c=MID),
            in1=mv[:, :, C1:T1C].rearrange("p r (h o) -> p r h o", o=1)
                .to_broadcast([128, R, HEADS, MID]),
            op=ALU.mult)

        pb = pp.tile([128, T1C], F32, space="PSUM", tag="pb")
        for r in range(R):
            nc.tensor.matmul(out=pb[:], lhsT=ident[:],
                             rhs=msg[:, r * T1C:(r + 1) * T1C],
                             start=(r == 0), stop=(r == R - 1))

        # post: normalize, bias, elu -> h1'' ; fused dense layer 2
        dn = sp.tile([128, HEADS], F32, tag="dn")
        nc.vector.tensor_scalar_add(out=dn[:], in0=pb[:, C1:T1C], scalar1=EPS)
        rc = sp.tile([128, HEADS], F32, tag="rc")
        nc.vector.reciprocal(out=rc[:], in_=dn[:])
        q = sp.tile([128, C1], F32, tag="q")
        nc.vector.tensor_tensor(
            out=q[:].rearrange("p (h c) -> p h c", c=MID),
            in0=pb[:, 0:C1].rearrange("p (h c) -> p h c", c=MID),
            in1=rc[:].rearrange("p (h o) -> p h o", o=1)
                .to_broadcast([128, HEADS, MID]),
            op=ALU.mult)
        z = sp.tile([128, C1], F32, tag="z")
        nc.vector.tensor_tensor(out=z[:], in0=q[:], in1=b1_sb[:], op=ALU.add)
        tmin = sp.tile([128, C1], F32, tag="tmin")
        nc.vector.tensor_scalar_min(out=tmin[:], in0=z[:], scalar1=0.0)
        u = sp.tile([128, C1], F32, tag="u")
        nc.scalar.activation(u[:], tmin[:], AF.Exp)
        d = sp.tile([128, C1], F32, tag="d")
        nc.vector.tensor_tensor(out=d[:], in0=z[:], in1=tmin[:], op=ALU.subtract)
        hp = sp.tile([128, C1], F32, tag="hp")
        nc.vector.tensor_tensor(out=hp[:], in0=d[:], in1=u[:], op=ALU.add)
        hb = sp.tile([128, C1], BF16, tag="hb")
        nc.vector.tensor_scalar_add(out=hb[:], in0=hp[:], scalar1=-1.0)

        pt = pp.tile([C1, 128], BF16, space="PSUM", tag="pt", bufs=1)
        nc.tensor.transpose(out=pt[:], in_=hb[:], identity=ident[:])
        hT = sp.tile([C1, 128], BF16, tag="hT")
        nc.vector.tensor_copy(out=hT[:], in_=pt[:])

        p2 = pp.tile([128, T2C + HEADS], F32, space="PSUM", tag="p2", bufs=1)
        nc.tensor.matmul(out=p2[:], lhsT=hT[:], rhs=w2[:], start=True, stop=True)
        t2 = sp.tile([128, T2C], BF16, tag="t2")
        nc.vector.tensor_copy(out=t2[:], in_=p2[:, :T2C])
        nc.sync.dma_start(out=tab2_loc[b * 128:(b + 1) * 128, :], in_=t2[:])
        nc.vector.tensor_copy(out=ad2[:, b * HEADS:(b + 1) * HEADS],
                              in_=p2[:, T2C:T2C + HEADS])

    # ---- phase 4: AllGather table 2
    nc.gpsimd.collective_compute(
        "AllGather", ALU.bypass,
        replica_groups=[list(range(NCORES))],
        ins=[tab2_loc.ap().opt()],
        outs=[tab2.ap().opt()],
    )

    # ---- phase 5: edge layer 2 + head mean + log_softmax
    for b in range(NBLK):
        R = R_blocks[b]
        c0 = int(chunk0[b])
        nrows = min(128, PER - b * 128)

        G = bigp.tile([128, R * T2C], BF16, tag="bigshare")
        for r in range(R):
            nc.gpsimd.indirect_dma_start(
                out=G[:, r * T2C:(r + 1) * T2C],
                out_offset=None,
                in_=tab2[:, :],
                in_offset=bass.IndirectOffsetOnAxis(
                    ap=idx_sb[:, c0 + r:c0 + r + 1], axis=0),
            )
        Gv = G[:].rearrange("p (r x) -> p r x", x=T2C)

        ea = ep.tile([128, R * HEADS], F32, tag="ea")
        nc.vector.tensor_tensor(
            out=ea[:].rearrange("p (r h) -> p r h", h=HEADS),
            in0=Gv[:, :, C2:T2C],
            in1=ad2[:, b * HEADS:(b + 1) * HEADS]
                .rearrange("p (o h) -> p o h", o=1).to_broadcast([128, R, HEADS]),
            op=ALU.add)
        eb = ep.tile([128, R * HEADS], F32, tag="eb")
        nc.vector.tensor_tensor(
            out=eb[:].rearrange("p (r h) -> p r h", h=HEADS),
            in0=ea[:].rearrange("p (r h) -> p r h", h=HEADS),
            in1=am_sb[:, c0:c0 + R]
                .rearrange("p (r o) -> p r o", o=1).to_broadcast([128, R, HEADS]),
            op=ALU.add)
        y1 = ep.tile([128, R * HEADS], BF16, tag="y1")
        nc.scalar.activation(y1[:], eb[:], AF.Exp)
        y2 = ep.tile([128, R * HEADS], BF16, tag="y2")
        nc.scalar.activation(y2[:], eb[:], AF.Exp, scale=NEG_SLOPE)

        msg = mp.tile([128, R * T2C], BF16, tag="msg2")
        mv = msg[:].rearrange("p (r x) -> p r x", x=T2C)
        nc.vector.tensor_tensor(
            out=mv[:, :, C2:T2C],
            in0=y1[:].rearrange("p (r h) -> p r h", h=HEADS),
            in1=y2[:].rearrange("p (r h) -> p r h", h=HEADS),
            op=ALU.max)
        nc.vector.tensor_tensor(
            out=mv[:, :, 0:C2].rearrange("p r (h c) -> p r h c", c=OUT_C),
            in0=Gv[:, :, 0:C2].rearrange("p r (h c) -> p r h c", c=OUT_C),
            in1=mv[:, :, C2:T2C].rearrange("p r (h o) -> p r h o", o=1)
                .to_broadcast([128, R, HEADS, OUT_C]),
            op=ALU.mult)

        pb = pp.tile([128, T2C], F32, space="PSUM", tag="pb")
        for r in range(R):
            nc.tensor.matmul(out=pb[:], lhsT=ident[:],
                             rhs=msg[:, r * T2C:(r + 1) * T2C],
                             start=(r == 0), stop=(r == R - 1))

        dn = sp.tile([128, HEADS], F32, tag="dn")
        nc.vector.tensor_scalar_add(out=dn[:], in0=pb[:, C2:T2C], scalar1=EPS)
        rc = sp.tile([128, HEADS], F32, tag="rc")
        nc.vector.reciprocal(out=rc[:], in_=dn[:])
        q2 = sp.tile([128, C2], F32, tag="q2")
        nc.vector.tensor_tensor(
            out=q2[:].rearrange("p (h c) -> p h c", c=OUT_C),
            in0=pb[:, 0:C2].rearrange("p (h c) -> p h c", c=OUT_C),
            in1=rc[:].rearrange("p (h o) -> p h o", o=1)
                .to_broadcast([128, HEADS, OUT_C]),
            op=ALU.mult)
        s1 = sp.tile([128, C2 // 2], F32, tag="s1")
        nc.vector.tensor_tensor(out=s1[:], in0=q2[:, :64], in1=q2[:, 64:], op=ALU.add)
        s2 = sp.tile([128, C2 // 4], F32, tag="s2")
        nc.vector.tensor_tensor(out=s2[:], in0=s1[:, :32], in1=s1[:, 32:], op=ALU.add)
        s3 = sp.tile([128, OUT_C], F32, tag="s3")
        nc.vector.tensor_tensor(out=s3[:], in0=s2[:, :16], in1=s2[:, 16:], op=ALU.add)
        o1 = sp.tile([128, OUT_C], F32, tag="o1")
        nc.vector.tensor_scalar_mul(out=o1[:], in0=s3[:], scalar1=1.0 / HEADS)
        o2 = sp.tile([128, OUT_C], F32, tag="o2")
        nc.vector.tensor_tensor(out=o2[:], in0=o1[:], in1=b2_sb[:], op=ALU.add)

        eo = sp.tile([128, OUT_C], F32, tag="eo")
        nc.scalar.activation(eo[:], o2[:], AF.Exp,
                             accum_out=ls_all[:, b:b + 1])
        nc.vector.tensor_copy(out=o2_all[:, b * OUT_C:(b + 1) * OUT_C], in_=o2[:])

    ln_all = st.tile([128, NBLK], F32)
    nc.scalar.activation(ln_all[:], ls_all[:], AF.Ln)
    res_all = st.tile([128, NBLK * OUT_C], F32)
    nc.vector.tensor_tensor(
        out=res_all[:].rearrange("p (b c) -> p b c", c=OUT_C),
        in0=o2_all[:].rearrange("p (b c) -> p b c", c=OUT_C),
        in1=ln_all[:].rearrange("p (b o) -> p b o", o=1)
            .to_broadcast([128, NBLK, OUT_C]),
        op=ALU.subtract)
    nc.sync.dma_start(
        out=t_out.ap().rearrange("(b p) c -> p b c", p=128),
        in_=res_all[:].rearrange("p (b c) -> p b c", c=OUT_C))

    ctx.close()


# ------------------------------------------------------------------- driver

_CACHE = {}


def _get_program(key, R_blocks, TOT):
    if key not in _CACHE:
        _CACHE[key] = build_program(R_blocks, TOT)
    return _CACHE[key]


def kernel(node_feature, adj_list, W1, att_src1, att_dst1, b1,
           W2, att_src2, att_dst2, b2, _trace=False):
    prep = _host_prep(node_feature, adj_list, W1, att_src1, att_dst1, b1,
                      W2, att_src2, att_dst2, b2)
    R_blocks, TOT = prep["R_blocks"], prep["TOT"]
    nc = _get_program(tuple(R_blocks), R_blocks, TOT)

    in_maps = []
    for c in range(NCORES):
        in_maps.append({
            "xT": prep["xT_slices"][c],
            "W1e": prep["W1e"],
            "W2e": prep["W2e"],
            "sidx": prep["src_idx"][c],
            "amask": prep["amask"][c],
            "b1rep": prep["b1rep"],
            "b2rep": prep["b2rep"],
        })
    res = bass_utils.run_bass_kernel_spmd(
        nc, in_maps, list(range(NCORES)), trace=_trace)

    y_new = np.concatenate([res.results[c]["y"] for c in range(NCORES)], axis=0)
    out = y_new[prep["new_of_node"]]
    if _trace:
        kernel._last_result = res
    return np.ascontiguousarray(out.astype(np.float32))


# revision 10
# speedup vs baseline: 1.0605x; 1.0168x over previous
"""Trainium2 Bass kernel for a 2-layer GAT (nn_GAT_48524540510808).

Strategy (8 NeuronCores, SPMD):
- Nodes permuted by in-degree (desc) and dealt round-robin across cores:
  global rank k -> core k%8, local slot k//8; new node id = core*6250 + local.
- Per core, 49 blocks of 128 local nodes. Block b is processed in R_b rounds
  (R_b = degree at global rank 1024*b); round r slot p holds the r-th in-edge
  of local node 128*b+p (pad slots are masked with an additive -30000 on the
  attention logit). Segment-sum therefore becomes plain PSUM accumulation of
  per-round message tiles via an identity matmul (no scatter needed).
- Dense projections are data-parallel over nodes; per-layer tables
  (h | alpha_src) in bf16 are AllGathered so each core can gather rows of any
  src node. alpha_dst stays core-local in SBUF.
- Edge phase gathers 128 rows per round with one indirect DMA.
- Softmax over incoming edges is computed without segment-max
  (exp(leakyrelu(e)) = max(exp(e), exp(0.28 e)); logits are O(1) so direct
  exp is safe in fp32/bf16). Normalization happens after aggregation:
  out = (sum_e w_e h_src) / (sum_e w_e + eps).
- elu(z) = relu(z) + exp(min(z,0)) - 1; log_softmax via exp-with-accum + ln.
"""

import numpy as np
import ml_dtypes

import concourse.bass as bass
import concourse.mybir as mybir
import concourse.tile as tile
import concourse.bacc as bacc
import concourse.bass_utils as bass_utils

bf16 = ml_dtypes.bfloat16

N = 50000
E = 800000
IN_C = 512
MID = 8
HEADS = 8
OUT_C = 16
NEG_SLOPE = 0.28
EPS = 1e-16
NCORES = 8
PER = N // NCORES            # 6250
NBLK = (PER + 127) // 128    # 49
PER_PAD = NBLK * 128         # 6272
KIN = IN_C // 128            # 4

C1 = HEADS * MID             # 64  (layer-1 h channels)
T1C = C1 + HEADS             # 72  (psum cols: h | alpha_src)
C2 = HEADS * OUT_C           # 128 (layer-2 h channels)
T2C = C2 + HEADS             # 136 (psum cols: h2 | alpha_src2)
T1B = C1 + 2 * HEADS         # 80  fp8-elem bytes per tab1 row (h fp8 | as bf16)
T2B = C2 + 2 * HEADS         # 144 fp8-elem bytes per tab2 row

F32 = mybir.dt.float32
BF16 = mybir.dt.bfloat16
FP8 = mybir.dt.float8e4
I32 = mybir.dt.int32
AF = mybir.ActivationFunctionType
ALU = mybir.AluOpType


# ---------------------------------------------------------------- host prep

def _host_prep(node_feature, adj_list, W1, att_src1, att_dst1, b1,
               W2, att_src2, att_dst2, b2):
    src = np.asarray(adj_list[0], np.int64)
    dst = np.asarray(adj_list[1], np.int64)

    deg = np.bincount(dst, minlength=N)
    order = np.argsort(-deg, kind="stable")          # rank -> node
    rank_of_node = np.empty(N, np.int64)
    rank_of_node[order] = np.arange(N)
    ranks = np.arange(N)
    new_of_rank = (ranks % NCORES) * PER_PAD + (ranks // NCORES)
    new_of_node = new_of_rank[rank_of_node]          # node -> new id

    deg_sorted = deg[order]
    R_blocks = [int(deg_sorted[1024 * b]) for b in range(NBLK)]
    R_blocks = [max(r, 1) for r in R_blocks]
    chunk0 = np.concatenate([[0], np.cumsum(R_blocks)]).astype(np.int64)
    TOT = int(chunk0[-1])

    ns, nd = new_of_node[src], new_of_node[dst]
    eorder = np.lexsort((ns, nd))
    nd_s, ns_s = nd[eorder], ns[eorder]
    grp_start = np.searchsorted(nd_s, np.arange(NCORES * PER_PAD), side="left")
    pos = np.arange(E) - grp_start[nd_s]
    core_e = nd_s // PER_PAD
    loc_e = nd_s % PER_PAD
    blk_e = loc_e // 128
    part_e = loc_e % 128
    chunk_e = chunk0[blk_e] + pos

    src_idx = np.zeros((NCORES, 128, TOT), np.int32)
    amask = np.full((NCORES, 128, TOT), -30000.0, np.float32)
    src_idx[core_e, part_e, chunk_e] = ns_s.astype(np.int32)
    amask[core_e, part_e, chunk_e] = 0.0

    # folded weights
    A1 = np.zeros((C1, 2 * HEADS), np.float32)
    for h in range(HEADS):
        A1[h * MID:(h + 1) * MID, h] = att_src1[h]
        A1[h * MID:(h + 1) * MID, HEADS + h] = att_dst1[h]
    W1e = np.concatenate([W1, W1 @ A1], axis=1).astype(bf16)     # [512, 80]
    A2 = np.zeros((C2, 2 * HEADS), np.float32)
    for h in range(HEADS):
        A2[h * OUT_C:(h + 1) * OUT_C, h] = att_src2[h]
        A2[h * OUT_C:(h + 1) * OUT_C, HEADS + h] = att_dst2[h]
    W2e = np.concatenate([W2, W2 @ A2], axis=1).astype(bf16)     # [64, 160]

    xf = np.asarray(node_feature)
    xT_slices = []
    for c in range(NCORES):
        nodes_c = order[c::NCORES]                   # local l -> node
        s = np.zeros((IN_C, PER_PAD), bf16)
        s[:, :PER] = xf[nodes_c].T.astype(bf16)
        xT_slices.append(s)

    b1rep = np.tile(np.asarray(b1, np.float32)[None, :], (128, 1))
    b2rep = np.tile(np.asarray(b2, np.float32)[None, :], (128, 1))

    return dict(
        R_blocks=R_blocks, chunk0=chunk0, TOT=TOT,
        src_idx=src_idx, amask=amask,
        W1e=np.asarray(W1e), W2e=np.asarray(W2e),
        xT_slices=xT_slices, b1rep=b1rep, b2rep=b2rep,
        new_of_node=new_of_node,
    )


# ------------------------------------------------------------- bass program

def build_program(R_blocks, TOT):
    nc = bacc.Bacc("TRN2", num_devices=NCORES)

    t_xT = nc.dram_tensor("xT", [IN_C, PER_PAD], BF16, kind="ExternalInput")
    t_W1e = nc.dram_tensor("W1e", [IN_C, T1C + HEADS], BF16, kind="ExternalInput")
    t_W2e = nc.dram_tensor("W2e", [C1, T2C + HEADS], BF16, kind="ExternalInput")
    t_idx = nc.dram_tensor("sidx", [128, TOT], I32, kind="ExternalInput")
    t_amask = nc.dram_tensor("amask", [128, TOT], F32, kind="ExternalInput")
    t_b1 = nc.dram_tensor("b1rep", [128, C1], F32, kind="ExternalInput")
    t_b2 = nc.dram_tensor("b2rep", [128, OUT_C], F32, kind="ExternalInput")
    t_out = nc.dram_tensor("y", [PER_PAD, OUT_C], F32, kind="ExternalOutput")

    tab1_loc = nc.dram_tensor("tab1_loc", [PER_PAD, T1B], FP8)
    tab2_loc = nc.dram_tensor("tab2_loc", [PER_PAD, T2B], FP8)
    tab1 = nc.dram_tensor("tab1", [NCORES * PER_PAD, T1B], FP8, addr_space="Shared")
    tab2 = nc.dram_tensor("tab2", [NCORES * PER_PAD, T2B], FP8, addr_space="Shared")

    chunk0 = np.concatenate([[0], np.cumsum(R_blocks)]).astype(np.int64)

    with tile.TileContext(nc) as tc:
        _emit(tc, nc, R_blocks, chunk0, TOT,
              t_xT, t_W1e, t_W2e, t_idx, t_amask, t_b1, t_b2, t_out,
              tab1_loc, tab2_loc, tab1, tab2)
    nc.compile()
    return nc


def _emit(tc, nc, R_blocks, chunk0, TOT,
          t_xT, t_W1e, t_W2e, t_idx, t_amask, t_b1, t_b2, t_out,
          tab1_loc, tab2_loc, tab1, tab2):
    from concourse.masks import make_identity

    from contextlib import ExitStack
    ctx = ExitStack()
    st = ctx.enter_context(tc.tile_pool(name="static", bufs=1))
    bigp = ctx.enter_context(tc.tile_pool(name="bigp", bufs=5))
    gp = ctx.enter_context(tc.tile_pool(name="gp", bufs=7))
    mp = ctx.enter_context(tc.tile_pool(name="mp", bufs=3))
    ep = ctx.enter_context(tc.tile_pool(name="ep", bufs=3))
    pp = ctx.enter_context(tc.tile_pool(name="pp", bufs=2, space="PSUM"))
    sp = ctx.enter_context(tc.tile_pool(name="sp", bufs=3))

    ident = st.tile([128, 128], BF16)
    make_identity(nc, ident[:])

    # static SBUF loads
    w1t = [st.tile([128, T1C + HEADS], BF16, name=f"w1_{k}", tag=f"w1_{k}") for k in range(KIN)]
    for k in range(KIN):
        nc.sync.dma_start(out=w1t[k][:], in_=t_W1e[k * 128:(k + 1) * 128, :])
    w2 = st.tile([C1, T2C + HEADS], BF16)
    nc.sync.dma_start(out=w2[:], in_=t_W2e[:, :])

    idx_sb = st.tile([128, TOT], I32)
    nc.sync.dma_start(out=idx_sb[:], in_=t_idx[:, :])
    am_sb = st.tile([128, TOT], F32)
    nc.sync.dma_start(out=am_sb[:], in_=t_amask[:, :])
    b1_sb = st.tile([128, C1], F32)
    nc.sync.dma_start(out=b1_sb[:], in_=t_b1[:, :])
    b2_sb = st.tile([128, OUT_C], F32)
    nc.sync.dma_start(out=b2_sb[:], in_=t_b2[:, :])

    ad1 = st.tile([128, NBLK * HEADS], BF16)
    ad2 = st.tile([128, NBLK * HEADS], BF16)
    tb_all = st.tile([128, NBLK * T1B], FP8)
    ls_all = st.tile([128, NBLK], F32)
    o2_all = st.tile([128, NBLK * OUT_C], F32)

    # ---- phase 1: dense layer 1 (data-parallel over this core's nodes)
    xbig = []
    for k in range(KIN):
        xb = bigp.tile([128, PER_PAD], BF16, name=f"xb{k}", tag="bigshare")
        nc.sync.dma_start(out=xb[:], in_=t_xT[k * 128:(k + 1) * 128, :])
        xbig.append(xb)
    for m in range(NBLK):
        pd = pp.tile([128, T1C + HEADS], F32, space="PSUM", tag="pd")
        for k in range(KIN):
            nc.tensor.matmul(out=pd[:], lhsT=xbig[k][:, m * 128:m * 128 + 128],
                             rhs=w1t[k][:],
                             start=(k == 0), stop=(k == KIN - 1))
        nc.vector.tensor_copy(out=tb_all[:, m * T1B:m * T1B + C1], in_=pd[:, :C1])
        nc.vector.tensor_copy(
            out=tb_all[:, m * T1B + C1:(m + 1) * T1B].bitcast(BF16),
            in_=pd[:, C1:T1C])
        nc.vector.tensor_copy(out=ad1[:, m * HEADS:(m + 1) * HEADS],
                              in_=pd[:, T1C:T1C + HEADS])
    nc.sync.dma_start(
        out=tab1_loc.ap().rearrange("(b p) c -> p b c", p=128),
        in_=tb_all[:].rearrange("p (b c) -> p b c", c=T1B))

    # ---- phase 2: AllGather table 1
    nc.gpsimd.collective_compute(
        "AllGather", ALU.bypass,
        replica_groups=[list(range(NCORES))],
        ins=[tab1_loc.ap().opt()],
        outs=[tab1.ap().opt()],
    )

    # ---- phase 3: edge layer 1 (+fused dense layer 2 per block)
    for b in range(NBLK):
        R = R_blocks[b]
        c0 = int(chunk0[b])
        nrows = min(128, PER - b * 128)

        G = gp.tile([128, R * T1B], FP8, tag="G")
        for r in range(R):
            nc.gpsimd.indirect_dma_start(
                out=G[:, r * T1B:(r + 1) * T1B],
                out_offset=None,
                in_=tab1[:, :],
                in_offset=bass.IndirectOffsetOnAxis(
                    ap=idx_sb[:, c0 + r:c0 + r + 1], axis=0),
            )
        Gv = G[:].rearrange("p (r x) -> p r x", x=T1B)

        ea = ep.tile([128, R * HEADS], F32, tag="ea")
        nc.vector.tensor_tensor(
            out=ea[:].rearrange("p (r h) -> p r h", h=HEADS),
            in0=Gv[:, :, C1:T1B].bitcast(BF16),
            in1=ad1[:, b * HEADS:(b + 1) * HEADS]
                .rearrange("p (o h) -> p o h", o=1).to_broadcast([128, R, HEADS]),
            op=ALU.add)
        eb = ep.tile([128, R * HEADS], F32, tag="eb")
        nc.vector.tensor_tensor(
            out=eb[:].rearrange("p (r h) -> p r h", h=HEADS),
            in0=ea[:].rearrange("p (r h) -> p r h", h=HEADS),
            in1=am_sb[:, c0:c0 + R]
                .rearrange("p (r o) -> p r o", o=1).to_broadcast([128, R, HEADS]),
            op=ALU.add)
        y1 = ep.tile([128, R * HEADS], BF16, tag="y1")
        nc.scalar.activation(y1[:], eb[:], AF.Exp)
        y2 = ep.tile([128, R * HEADS], BF16, tag="y2")
        nc.scalar.activation(y2[:], eb[:], AF.Exp, scale=NEG_SLOPE)

        msg = mp.tile([128, R * T1C], BF16, tag="msg")
        mv = msg[:].rearrange("p (r x) -> p r x", x=T1C)
        nc.vector.tensor_tensor(
            out=mv[:, :, C1:T1C],
            in0=y1[:].rearrange("p (r h) -> p r h", h=HEADS),
            in1=y2[:].rearrange("p (r h) -> p r h", h=HEADS),
            op=ALU.max)
        nc.vector.tensor_tensor(
            out=mv[:, :, 0:C1].rearrange("p r (h c) -> p r h c", c=MID),
            in0=Gv[:, :, 0:C1].rearrange("p r (h c) -> p r h c", c=MID),
            in1=mv[:, :, C1:T1C].rearrange("p r (h o) -> p r h o", o=1)
                .to_broadcast([128, R, HEADS, MID]),
            op=ALU.mult)

        pb = pp.tile([128, T1C], F32, space="PSUM", tag="pb")
        for r in range(R):
            nc.tensor.matmul(out=pb[:], lhsT=ident[:],
                             rhs=msg[:, r * T1C:(r + 1) * T1C],
                             start=(r == 0), stop=(r == R - 1))

        # post: normalize, bias, elu -> h1'' ; fused dense layer 2
        dn = sp.tile([128, HEADS], F32, tag="dn")
        nc.vector.tensor_scalar_add(out=dn[:], in0=pb[:, C1:T1C], scalar1=EPS)
        rc = sp.tile([128, HEADS], F32, tag="rc")
        nc.vector.reciprocal(out=rc[:], in_=dn[:])
        q = sp.tile([128, C1], F32, tag="q")
        nc.vector.tensor_tensor(
            out=q[:].rearrange("p (h c) -> p h c", c=MID),
            in0=pb[:, 0:C1].rearrange("p (h c) -> p h c", c=MID),
            in1=rc[:].rearrange("p (h o) -> p h o", o=1)
                .to_broadcast([128, HEADS, MID]),
            op=ALU.mult)
        z = sp.tile([128, C1], F32, tag="z")
        nc.vector.tensor_tensor(out=z[:], in0=q[:], in1=b1_sb[:], op=ALU.add)
        tmin = sp.tile([128, C1], F32, tag="tmin")
        nc.vector.tensor_scalar_min(out=tmin[:], in0=z[:], scalar1=0.0)
        u = sp.tile([128, C1], F32, tag="u")
        nc.scalar.activation(u[:], tmin[:], AF.Exp)
        d = sp.tile([128, C1], F32, tag="d")
        nc.vector.tensor_tensor(out=d[:], in0=z[:], in1=tmin[:], op=ALU.subtract)
        hp = sp.tile([128, C1], F32, tag="hp")
        nc.vector.tensor_tensor(out=hp[:], in0=d[:], in1=u[:], op=ALU.add)
        hb = sp.tile([128, C1], BF16, tag="hb")
        nc.vector.tensor_scalar_add(out=hb[:], in0=hp[:], scalar1=-1.0)

        pt = pp.tile([C1, 128], BF16, space="PSUM", tag="pt", bufs=1)
        nc.tensor.transpose(out=pt[:], in_=hb[:], identity=ident[:])
        hT = sp.tile([C1, 128], BF16, tag="hT")
        nc.vector.tensor_copy(out=hT[:], in_=pt[:])

        p2 = pp.tile([128, T2C + HEADS], F32, space="PSUM", tag="p2", bufs=1)
        nc.tensor.matmul(out=p2[:], lhsT=hT[:], rhs=w2[:], start=True, stop=True)
        t2 = sp.tile([128, T2B], FP8, tag="t2")
        nc.vector.tensor_copy(out=t2[:, :C2], in_=p2[:, :C2])
        nc.vector.tensor_copy(out=t2[:, C2:T2B].bitcast(BF16), in_=p2[:, C2:T2C])
        nc.sync.dma_start(out=tab2_loc[b * 128:(b + 1) * 128, :], in_=t2[:])
        nc.vector.tensor_copy(out=ad2[:, b * HEADS:(b + 1) * HEADS],
                              in_=p2[:, T2C:T2C + HEADS])

    # ---- phase 4: AllGather table 2
    nc.gpsimd.collective_compute(
        "AllGather", ALU.bypass,
        replica_groups=[list(range(NCORES))],
        ins=[tab2_loc.ap().opt()],
        outs=[tab2.ap().opt()],
    )

    # ---- phase 5: edge layer 2 + head mean + log_softmax
    for b in range(NBLK):
        R = R_blocks[b]
        c0 = int(chunk0[b])
        nrows = min(128, PER - b * 128)

        G = bigp.tile([128, R * T2B], FP8, tag="bigshare")
        for r in range(R):
            nc.gpsimd.indirect_dma_start(
                out=G[:, r * T2B:(r + 1) * T2B],
                out_offset=None,
                in_=tab2[:, :],
                in_offset=bass.IndirectOffsetOnAxis(
                    ap=idx_sb[:, c0 + r:c0 + r + 1], axis=0),
            )
        Gv = G[:].rearrange("p (r x) -> p r x", x=T2B)

        ea = ep.tile([128, R * HEADS], F32, tag="ea")
        nc.vector.tensor_tensor(
            out=ea[:].rearrange("p (r h) -> p r h", h=HEADS),
            in0=Gv[:, :, C2:T2B].bitcast(BF16),
            in1=ad2[:, b * HEADS:(b + 1) * HEADS]
                .rearrange("p (o h) -> p o h", o=1).to_broadcast([128, R, HEADS]),
            op=ALU.add)
        eb = ep.tile([128, R * HEADS], F32, tag="eb")
        nc.vector.tensor_tensor(
            out=eb[:].rearrange("p (r h) -> p r h", h=HEADS),
            in0=ea[:].rearrange("p (r h) -> p r h", h=HEADS),
            in1=am_sb[:, c0:c0 + R]
                .rearrange("p (r o) -> p r o", o=1).to_broadcast([128, R, HEADS]),
            op=ALU.add)
        y1 = ep.tile([128, R * HEADS], BF16, tag="y1")
        nc.scalar.activation(y1[:], eb[:], AF.Exp)
        y2 = ep.tile([128, R * HEADS], BF16, tag="y2")
        nc.scalar.activation(y2[:], eb[:], AF.Exp, scale=NEG_SLOPE)

        msg = mp.tile([128, R * T2C], BF16, tag="msg2")
        mv = msg[:].rearrange("p (r x) -> p r x", x=T2C)
        nc.vector.tensor_tensor(
            out=mv[:, :, C2:T2C],
            in0=y1[:].rearrange("p (r h) -> p r h", h=HEADS),
            in1=y2[:].rearrange("p (r h) -> p r h", h=HEADS),
            op=ALU.max)
        nc.vector.tensor_tensor(
            out=mv[:, :, 0:C2].rearrange("p r (h c) -> p r h c", c=OUT_C),
            in0=Gv[:, :, 0:C2].rearrange("p r (h c) -> p r h c", c=OUT_C),
            in1=mv[:, :, C2:T2C].rearrange("p r (h o) -> p r h o", o=1)
                .to_broadcast([128, R, HEADS, OUT_C]),
            op=ALU.mult)

        pb = pp.tile([128, T2C], F32, space="PSUM", tag="pb")
        for r in range(R):
            nc.tensor.matmul(out=pb[:], lhsT=ident[:],
                             rhs=msg[:, r * T2C:(r + 1) * T2C],
                             start=(r == 0), stop=(r == R - 1))

        dn = sp.tile([128, HEADS], F32, tag="dn")
        nc.vector.tensor_scalar_add(out=dn[:], in0=pb[:, C2:T2C], scalar1=EPS)
        rc = sp.tile([128, HEADS], F32, tag="rc")
        nc.vector.reciprocal(out=rc[:], in_=dn[:])
        q2 = sp.tile([128, C2], F32, tag="q2")
        nc.vector.tensor_tensor(
            out=q2[:].rearrange("p (h c) -> p h c", c=OUT_C),
            in0=pb[:, 0:C2].rearrange("p (h c) -> p h c", c=OUT_C),
            in1=rc[:].rearrange("p (h o) -> p h o", o=1)
                .to_broadcast([128, HEADS, OUT_C]),
            op=ALU.mult)
        s1 = sp.tile([128, C2 // 2], F32, tag="s1")
        nc.vector.tensor_tensor(out=s1[:], in0=q2[:, :64], in1=q2[:, 64:], op=ALU.add)
        s2 = sp.tile([128, C2 // 4], F32, tag="s2")
        nc.vector.tensor_tensor(out=s2[:], in0=s1[:, :32], in1=s1[:, 32:], op=ALU.add)
        s3 = sp.tile([128, OUT_C], F32, tag="s3")
        nc.vector.tensor_tensor(out=s3[:], in0=s2[:, :16], in1=s2[:, 16:], op=ALU.add)
        o1 = sp.tile([128, OUT_C], F32, tag="o1")
        nc.vector.tensor_scalar_mul(out=o1[:], in0=s3[:], scalar1=1.0 / HEADS)
        o2 = sp.tile([128, OUT_C], F32, tag="o2")
        nc.vector.tensor_tensor(out=o2[:], in0=o1[:], in1=b2_sb[:], op=ALU.add)

        eo = sp.tile([128, OUT_C], F32, tag="eo")
        nc.scalar.activation(eo[:], o2[:], AF.Exp,
                             accum_out=ls_all[:, b:b + 1])
        nc.vector.tensor_copy(out=o2_all[:, b * OUT_C:(b + 1) * OUT_C], in_=o2[:])

    ln_all = st.tile([128, NBLK], F32)
    nc.scalar.activation(ln_all[:], ls_all[:], AF.Ln)
    res_all = st.tile([128, NBLK * OUT_C], F32)
    nc.vector.tensor_tensor(
        out=res_all[:].rearrange("p (b c) -> p b c", c=OUT_C),
        in0=o2_all[:].rearrange("p (b c) -> p b c", c=OUT_C),
        in1=ln_all[:].rearrange("p (b o) -> p b o", o=1)
            .to_broadcast([128, NBLK, OUT_C]),
        op=ALU.subtract)
    nc.sync.dma_start(
        out=t_out.ap().rearrange("(b p) c -> p b c", p=128),
        in_=res_all[:].rearrange("p (b c) -> p b c", c=OUT_C))

    ctx.close()


# ------------------------------------------------------------------- driver

_CACHE = {}


def _get_program(key, R_blocks, TOT):
    if key not in _CACHE:
        _CACHE[key] = build_program(R_blocks, TOT)
    return _CACHE[key]


def kernel(node_feature, adj_list, W1, att_src1, att_dst1, b1,
           W2, att_src2, att_dst2, b2, _trace=False):
    prep = _host_prep(node_feature, adj_list, W1, att_src1, att_dst1, b1,
                      W2, att_src2, att_dst2, b2)
    R_blocks, TOT = prep["R_blocks"], prep["TOT"]
    nc = _get_program(tuple(R_blocks), R_blocks, TOT)

    in_maps = []
    for c in range(NCORES):
        in_maps.append({
            "xT": prep["xT_slices"][c],
            "W1e": prep["W1e"],
            "W2e": prep["W2e"],
            "sidx": prep["src_idx"][c],
            "amask": prep["amask"][c],
            "b1rep": prep["b1rep"],
            "b2rep": prep["b2rep"],
        })
    res = bass_utils.run_bass_kernel_spmd(
        nc, in_maps, list(range(NCORES)), trace=_trace)

    y_new = np.concatenate([res.results[c]["y"] for c in range(NCORES)], axis=0)
    out = y_new[prep["new_of_node"]]
    if _trace:
        kernel._last_result = res
    return np.ascontiguousarray(out.astype(np.float32))
